# revision 53
# baseline (speedup 1.0000x reference)
"""Trainium2 Bass kernel for a dense transformer block (attention + GeGLU-mish
FFN) on x:[2,2048,768], distributed over 8 NeuronCores.

Sharding: core i handles batch i//4, query-block i%4 (512 rows). K/V for the
full 2048-token batch sequence are computed redundantly per core from the full
(host-rotated, bf16) x — no collectives, so the tensor engine never stalls on
an all-gather. All activations are feature-major (D on partitions) so every
matmul's contraction dim lands on partitions with no on-device transposes. The
host rotates each core's sequence so its own query block is always block 0
(attention is permutation-invariant over keys), letting all cores run one
identical SPMD program.
"""
import sys

sys.path.insert(0, "/opt/trn_rl_repo")

import numpy as np
import ml_dtypes

import bass_rust
import concourse.bass as bass
import concourse.mybir as mybir
import concourse.tile as tile
from concourse.bass_utils import run_bass_kernel_spmd

AF = mybir.ActivationFunctionType
ALU = mybir.AluOpType
BF16 = mybir.dt.bfloat16
F32 = mybir.dt.float32
F32R = mybir.dt.float32r
F8 = mybir.dt.float8e4
DR = mybir.MatmulPerfMode.DoubleRow

USE_RSQRT = False  # hw Rsqrt is blocked in this bass build (accuracy issues)

# fp8 weight pre-scales (host-folded; compensated on-device). e4m3 denormals
# start at ~0.0019 and the raw weights are ~N(0, 0.0025..0.02), so scale them
# into the normal range and divide back out later.
AQK = 64.0   # q/k weight scale; scores scale by AQK^2, folded into exp()
AV = 8.0     # v weight scale; folded into the softmax denominator ones-col
A2 = 16.0    # w2 scale; folded into the final residual add
AG = 16.0    # w1-gate scale; folded into w2 (gate path is linear)
# mish(x) ~= SIG_C * x * sigmoid(SIG_A*x + SIG_B): one Activation op instead
# of the exact 5-op exp chain; SIG_C folds into w2 on host
SIG_A = 1.2571
SIG_B = 0.4654
SIG_C = 1.0113

DIM = 768
NH = 12
HD = 64
HIDDEN = 3072
S = 2048
QB = 512          # query rows per core
EPS = 1e-5
NCK = DIM // 128  # 6 chunks of the model dim
NCB = S // QB     # 4 column blocks of the sequence

# ---------------------------------------------------------------------------
# Workaround for a walrus codegen limit: an instruction may carry at most one
# sync-wait command, but TileContext's exit drain accumulates one wait per
# logical proc. Split the waits onto chained SP-engine NOPs before the drain.
# ---------------------------------------------------------------------------


def _split_waits(nc):
    """Hoist excess per-instruction sem waits onto same-engine NOPs (this
    walrus build accepts at most 1 wait on CTRL ops / 2 on compute ops)."""
    for f in nc.m.functions:
        for bb in f.blocks:
            snapshot = list(bb.instructions)
            new = []
            for inst in snapshot:
                si = inst.sync_info
                waits = list(si.on_wait) if si and si.on_wait else []
                limit = 1
                if len(waits) > limit:
                    si.on_wait = waits[:limit]
                    eng = nc.engines[inst.engine]
                    for w in waits[limit:]:
                        nop = eng.nop()
                        popped = nc.cur_bb.bb.instructions.pop()
                        assert popped is nop.ins
                        nop.ins.sync_info = bass_rust.SyncInfo(
                            on_wait=[w], on_update=[])
                        new.append(nop.ins)
                new.append(inst)
            bb.instructions[:] = new


# ---------------------------------------------------------------------------
# Device program
# ---------------------------------------------------------------------------


def build_nc(repeat=1):
    nc = bass.Bass()
    xb_d = nc.dram_tensor("xb", [DIM, S], BF16, kind="ExternalInput")
    # wqkv fp8 (as uint8 bytes: F8E4M3FN host arrays transfer pathologically
    # slowly through PJRT), d-chunk-pair layout: [p,m,s,col]=w[(2m+s)*128+p,col]
    wqkv_d = nc.dram_tensor("wqkv8", [128, 3, 2, 3 * DIM], mybir.dt.uint8,
                            kind="ExternalInput")
    wout_d = nc.dram_tensor("wout", [DIM, DIM], BF16, kind="ExternalInput")
    # w1 h-half pre-tiled on host: [24, 128, 6, 128] (col-chunk, p, d-chunk,
    # col); gate half as fp8 bytes in d-chunk-pair layout
    w1t_d = nc.dram_tensor("w1t", [24, 128, NCK, 128], BF16, kind="ExternalInput")
    w1g_d = nc.dram_tensor("w1g8", [128, 3, 2, HIDDEN], mybir.dt.uint8,
                           kind="ExternalInput")
    # w2 fp8, hidden-chunk-pair layout: [p, mj, s, col] = w2[(2mj+s)*128+p, col]
    w2_d = nc.dram_tensor("w28", [128, 12, 2, DIM], mybir.dt.uint8,
                          kind="ExternalInput")
    yT_d = nc.dram_tensor("yT", [DIM, QB], BF16, kind="ExternalOutput")

    with tile.TileContext(nc) as tc:
        for _ in range(repeat):
            _body(nc, tc, xb_d, wqkv_d, wout_d, w1t_d, w1g_d, w2_d, yT_d)
    _split_waits(nc)
    return nc


def _body(nc, tc, xb_d, wqkv_d, wout_d, w1t_d, w1g_d, w2_d, yT_d):
    from contextlib import ExitStack

    ctx = ExitStack()
    with ctx:
        singles = ctx.enter_context(tc.tile_pool(name="singles", bufs=1))

        ones_f = singles.tile([128, 1], F32)
        nc.vector.memset(ones_f[:], 1.0)
        ones = singles.tile([128, 1], F32R)
        nc.vector.tensor_copy(ones[:], ones_f[:])
        ones_bf = singles.tile([128, 1], BF16)
        nc.vector.tensor_copy(ones_bf[:], ones_f[:])
        eps1 = singles.tile([1, 1], F32)
        nc.vector.memset(eps1[:], EPS)
        sigb = singles.tile([128, 1], F32)
        nc.vector.memset(sigb[:], SIG_B)
        m_all_f = singles.tile([1, 128], F32)
        nc.vector.memset(m_all_f[:], 1.0)
        m_all = singles.tile([1, 128], F32R)
        nc.vector.tensor_copy(m_all[:], m_all_f[:])
        m_lo_f = singles.tile([1, 128], F32)
        nc.vector.memset(m_lo_f[:], 0.0)
        nc.vector.memset(m_lo_f[0:1, 0:64], 1.0)
        m_lo = singles.tile([1, 128], F32R)
        nc.vector.tensor_copy(m_lo[:], m_lo_f[:])
        m_hi_f = singles.tile([1, 128], F32)
        nc.vector.memset(m_hi_f[:], 0.0)
        nc.vector.memset(m_hi_f[0:1, 64:128], 1.0)
        m_hi = singles.tile([1, 128], F32R)
        nc.vector.tensor_copy(m_hi[:], m_hi_f[:])

        # persistent activations / weights (DMA emitted in ph12 below, in the
        # order compute consumes it: x cb0, wqkv q-cols, x cb1, k-cols, ...)
        x_full = [singles.tile([128, S], BF16, name=f"xf_{c}") for c in range(NCK)]
        x1 = [singles.tile([128, QB], F32, name=f"x1_{c}") for c in range(NCK)]
        xh1 = [singles.tile([128, QB], BF16, name=f"xh1_{c}") for c in range(NCK)]
        xh18 = singles.tile([128, NCK, QB], F8, name="xh18")
        wout_sb = [singles.tile([128, DIM], BF16, name=f"wout_{c}")
                   for c in range(NCK)]
        wv_sb = [singles.tile([128, QB], BF16, name=f"wv_{c}") for c in range(NCK)]

        # attention K/V/Q state (freed after attention so the FFN pools fit)
        p3_cm = tc.tile_pool(name="p3", bufs=1)
        p3 = p3_cm.__enter__()
        kT = [p3.tile([128, S], BF16, name=f"kT_{c}") for c in range(NCK)]
        qT = [p3.tile([128, QB], BF16, name=f"qT_{c}") for c in range(NCK)]
        # v in fp8, ktile-PAIR layout for DoubleRow wv matmuls. Per-head block
        # is padded to 96 cols (DR Ldweights needs a multiple of 32): 64 v
        # values, the AV denominator column, then zero pad.
        v2 = [p3.tile([128, 2, NH, 96], F8, name=f"v2_{m}")
              for m in range(8)]
        for m in range(8):
            nc.vector.memset(v2[m][:, :, :, HD + 1:96], 0.0)
        with tc.tile_pool(name="ph12", bufs=1) as ph12:
            wqkv_sb = ph12.tile([128, 3, 2, 3 * DIM], F8, name="wqkv8")

            def x_dma(cb):
                for c in range(NCK):
                    nc.sync.dma_start(
                        x_full[c][:, cb * QB:(cb + 1) * QB],
                        xb_d[c * 128:(c + 1) * 128, cb * QB:(cb + 1) * QB])

            def wqkv_dma(part):
                lo, hi = part * DIM, (part + 1) * DIM
                nc.sync.dma_start(wqkv_sb[:, :, :, lo:hi],
                                  wqkv_d[:, :, :, lo:hi].bitcast(F8))

            x_dma(0)
            wqkv_dma(0)          # q columns: needed first
            x_dma(1)
            wqkv_dma(1)          # k columns
            x_dma(2)
            wqkv_dma(2)          # v columns
            x_dma(3)
            for c in range(NCK):
                nc.sync.dma_start(wout_sb[c][:],
                                  wout_d[c * 128:(c + 1) * 128, :])
            # normalized activations in fp8, d-chunk-major (pairs of chunks
            # feed the DoubleRow projections)
            xh08 = ph12.tile([128, NCK, S], F8, name="xh08")

            def k_proj(oc, cb, pool, tag="kps"):
                """kT[oc] for column block cb (DR psum matmuls + DVE cast)."""
                lo, hi = cb * QB, (cb + 1) * QB
                ps = pool.tile([128, QB], F32, name=tag)
                for m in range(NCK // 2):
                    nc.tensor.matmul(
                        ps[:],
                        wqkv_sb[:, m, :, DIM + oc * 128:DIM + (oc + 1) * 128],
                        xh08[:, 2 * m:2 * m + 2, lo:hi],
                        start=(m == 0), stop=(m == NCK // 2 - 1),
                        perf_mode=DR)
                nc.vector.tensor_copy(kT[oc][:, lo:hi], ps[:])

            # ------------- phase 1+2: norm1, Q, K[0], V; cb-pipelined ------
            with tc.tile_pool(name="n1", bufs=3) as n1, \
                 tc.tile_pool(name="n1ps", bufs=1, space="PSUM") as n1ps, \
                 tc.tile_pool(name="n1bc", bufs=1, space="PSUM") as n1bc, \
                 tc.tile_pool(name="qkps", bufs=2, space="PSUM") as qkps, \
                 tc.tile_pool(name="vps", bufs=2, space="PSUM") as vps:
                for cb in range(NCB):
                    lo, hi = cb * QB, (cb + 1) * QB
                    # --- rmsnorm for this column block (sq mostly on the
                    # otherwise-idle Pool engine; DVE does the first chunks
                    # since the ss accumulation consumes them in order) ---
                    ss_ps = n1ps.tile([1, QB], F32, name="ss")
                    for c in range(NCK):
                        xt = x_full[c][:, lo:hi]
                        sq = n1.tile([128, QB], BF16, name="sq")
                        with nc.allow_low_precision(reason="x^2 in bf16, summed in f32 psum"):
                            nc.vector.tensor_tensor(sq[:], xt, xt, ALU.mult)
                        nc.tensor.matmul(ss_ps[:], ones_bf[:], sq[:],
                                         start=(c == 0), stop=(c == NCK - 1))
                    rstd = n1.tile([1, QB], F32R, name="rstd")
                    rstd0 = n1.tile([1, QB], F32, name="rstd0")
                    nc.scalar.activation(out=rstd0[:], in_=ss_ps[:],
                                         func=AF.Sqrt, bias=eps1[:],
                                         scale=1.0 / DIM)
                    with nc.allow_low_precision(reason="f32r==f32 bits"):
                        nc.vector.reciprocal(rstd[:], rstd0[:])
                    rbc = n1bc.tile([128, QB], F32, name="rbc")
                    nc.tensor.matmul(rbc[:], m_all[:], rstd[:],
                                     start=True, stop=True)
                    for c in range(NCK):
                        nc.vector.tensor_tensor(xh08[:, c, lo:hi],
                                                x_full[c][:, lo:hi],
                                                rbc[:], ALU.mult)

                    # --- q (own block only, cb == 0) ---
                    if cb == 0:
                        for oc in range(NCK):
                            ps = qkps.tile([128, QB], F32, name="qk")
                            for m in range(NCK // 2):
                                nc.tensor.matmul(
                                    ps[:],
                                    wqkv_sb[:, m, :, oc * 128:(oc + 1) * 128],
                                    xh08[:, 2 * m:2 * m + 2, 0:QB],
                                    start=(m == 0), stop=(m == NCK // 2 - 1),
                                    perf_mode=DR)
                            nc.scalar.copy(out=qT[oc][:], in_=ps[:])

                    # --- k chunk 0 only (rest interleaved with attention) ---
                    k_proj(0, cb, qkps, tag="qk")

                    # --- v (token-major, fp8 pair layout, + AV col) ---------
                    for t in range(4 * cb, 4 * cb + 4):
                        ps = vps.tile([128, DIM], F32, name="v")
                        for off, width in ((0, 512), (512, 256)):
                            for m in range(NCK // 2):
                                nc.tensor.matmul(
                                    ps[:, off:off + width],
                                    xh08[:, 2 * m:2 * m + 2,
                                         t * 128:(t + 1) * 128],
                                    wqkv_sb[:, m, :, 2 * DIM + off:
                                            2 * DIM + off + width],
                                    start=(m == 0), stop=(m == NCK // 2 - 1),
                                    perf_mode=DR)
                        # v cast on Act (idle until the exp stream starts)
                        nc.scalar.copy(
                            out=v2[t // 2][:, t % 2, :, 0:HD],
                            in_=ps.rearrange("p (h d) -> p h d", h=NH))
                        # the denominator column carries AV so the v-scale
                        # cancels in wv/den
                        nc.vector.memset(v2[t // 2][:, t % 2, :, HD:HD + 1],
                                         AV)

            # ------------- phase 3: attention, K[c] emitted just-in-time ---
            with tc.tile_pool(name="att_ps", bufs=2, space="PSUM") as ps_p, \
                 tc.tile_pool(name="att_wv", bufs=2, space="PSUM") as wv_p, \
                 tc.tile_pool(name="att_kr", bufs=2, space="PSUM") as kr_p, \
                 tc.tile_pool(name="att_pT", bufs=3) as pT_p, \
                 tc.tile_pool(name="att_den", bufs=2) as den_p:
                pend_norm = []
                # k for chunk 0 was produced in the cb loop; chunk c+1's
                # K-projection is interleaved INTO chunk c's kt loop (one
                # column block after every 4th ktile) so the tensor engine
                # fills the bubbles of the exp-paced attention stream.
                for c in range(NCK):
                    hA, hB = 2 * c, 2 * c + 1
                    wvA = wv_p.tile([128, QB], F32, name="wvps")
                    wvB = wv_p.tile([128, QB], F32, name="wvps")
                    for m in range(8):
                        pT4 = pT_p.tile([128, 2, 1024], F8, name="pT4")
                        for s_ in range(2):
                            kt = 2 * m + s_
                            if c + 1 < NCK and kt % 4 == 3:
                                k_proj(c + 1, kt // 4, kr_p)
                            kslc = kT[c][:, kt * 128:(kt + 1) * 128]
                            ps = ps_p.tile([128, 1024], F32, name="sAB")
                            nc.tensor.matmul(
                                ps[:, 0:QB], kslc[0:64, :],
                                qT[c][0:64, :], start=True, stop=True)
                            nc.tensor.matmul(
                                ps[:, QB:2 * QB], kslc[64:128, :],
                                qT[c][64:128, :], start=True, stop=True,
                                tile_position=(64, 0))
                            nc.scalar.activation(out=pT4[:, s_, :], in_=ps[:],
                                                 func=AF.Exp,
                                                 scale=1.0 / (AQK * AQK))
                        nc.tensor.matmul(
                            wvA[0:96, :], v2[m][:, :, hA, :],
                            pT4[:, :, 0:QB], start=(m == 0), stop=(m == 7),
                            perf_mode=DR)
                        nc.tensor.matmul(
                            wvB[0:96, :], v2[m][:, :, hB, :],
                            pT4[:, :, QB:2 * QB], start=(m == 0), stop=(m == 7),
                            perf_mode=DR)
                    # reciprocal denominators (f32r -> 1-cycle bcast matmuls)
                    denA = den_p.tile([1, QB], F32, name="denA")
                    nc.vector.tensor_copy(denA[:], wvA[HD:HD + 1, :])
                    denB = den_p.tile([1, QB], F32, name="denB")
                    nc.vector.tensor_copy(denB[:], wvB[HD:HD + 1, :])
                    recA = den_p.tile([1, QB], F32R, name="recA")
                    recB = den_p.tile([1, QB], F32R, name="recB")
                    with nc.allow_low_precision(reason="f32r==f32 bits"):
                        nc.vector.reciprocal(recA[:], denA[:])
                        nc.vector.reciprocal(recB[:], denB[:])
                    # stash unnormalized wv to SBUF (frees the psum tiles)
                    nc.vector.tensor_copy(wv_sb[c][0:64, :], wvA[0:HD, :])
                    nc.vector.tensor_copy(wv_sb[c][64:128, :], wvB[0:HD, :])
                    pend_norm.append((c, recA, recB))
                    # defer chunk c's normalize matmuls one chunk so the PE
                    # never waits on the DVE den/reciprocal chain
                    if len(pend_norm) > 1 or c == NCK - 1:
                        todo = (pend_norm if c == NCK - 1 else
                                [pend_norm.pop(0)])
                        for cc, rA, rB in todo:
                            rec = kr_p.tile([128, QB], F32, name="kps")
                            nc.tensor.matmul(rec[:], m_lo[:], rA[:],
                                             start=True, stop=False)
                            nc.tensor.matmul(rec[:], m_hi[:], rB[:],
                                             start=False, stop=True)
                            nc.vector.tensor_tensor(
                                wv_sb[cc][:], wv_sb[cc][:], rec[:], ALU.mult)
        p3_cm.__exit__(None, None, None)

        # ---------------- FFN weight pools (DMA overlaps out-proj/norm2) ---
        w1s = ctx.enter_context(tc.tile_pool(name="w1s", bufs=3))
        w2s = ctx.enter_context(tc.tile_pool(name="w2s", bufs=1))
        ffu = ctx.enter_context(tc.tile_pool(name="ffu", bufs=1))
        # u and w2 in fp8 hidden-chunk-pair layout for DoubleRow w2 matmuls
        u2 = [ffu.tile([128, 2, QB], F8, name=f"u2_{mj}") for mj in range(12)]
        w2_sb = [w2s.tile([128, 2, DIM], F8, name=f"w2_{mj}")
                 for mj in range(12)]
        for mj in range(12):
            nc.sync.dma_start(w2_sb[mj][:], w2_d[:, mj, :, :].bitcast(F8))
        w1g_sb = w2s.tile([128, 3, 2, HIDDEN], F8, name="w1g8")
        nc.sync.dma_start(w1g_sb[:], w1g_d[:, :, :, :].bitcast(F8))
        pre_w1 = {}
        for j in range(3):
            w1a = w1s.tile([128, NCK, 128], BF16, name="w1a")
            nc.sync.dma_start(w1a[:], w1t_d[j])
            pre_w1[j] = w1a

        # ---------------- phase 4: out-proj + residual + norm2 ----------
        with tc.tile_pool(name="op_ps", bufs=3, space="PSUM") as op_ps, \
             tc.tile_pool(name="n2ps", bufs=1, space="PSUM") as n2ps, \
             tc.tile_pool(name="n2bc", bufs=1, space="PSUM") as n2bc, \
             tc.tile_pool(name="n2", bufs=3) as n2:
            for oc in range(NCK):
                ps = op_ps.tile([128, QB], F32, name="op")
                for c in range(NCK):
                    nc.tensor.matmul(
                        ps[:], wout_sb[c][:, oc * 128:(oc + 1) * 128],
                        wv_sb[c][:], start=(c == 0), stop=(c == NCK - 1))
                nc.vector.tensor_tensor(x1[oc][:], ps[:],
                                        x_full[oc][:, 0:QB], ALU.add)
            # norm2
            ss_ps = n2ps.tile([1, QB], F32, name="ss2")
            for c in range(NCK):
                sq = n2.tile([128, QB], F32R, name="sq2")
                nc.vector.tensor_tensor(sq[:], x1[c][:], x1[c][:], ALU.mult)
                nc.tensor.matmul(ss_ps[:], ones[:], sq[:],
                                 start=(c == 0), stop=(c == NCK - 1))
            rstd = n2.tile([1, QB], F32R, name="rstd2")
            if USE_RSQRT:
                nc.scalar.activation(out=rstd[:], in_=ss_ps[:], func=AF.Rsqrt,
                                     bias=eps1[:], scale=1.0 / DIM)
            else:
                rstd0 = n2.tile([1, QB], F32, name="rstd20")
                nc.scalar.activation(out=rstd0[:], in_=ss_ps[:], func=AF.Sqrt,
                                     bias=eps1[:], scale=1.0 / DIM)
                with nc.allow_low_precision(reason="f32r==f32 bits"):
                    nc.vector.reciprocal(rstd[:], rstd0[:])
            rbc = n2bc.tile([128, QB], F32, name="rbc2")
            nc.tensor.matmul(rbc[:], m_all[:], rstd[:], start=True, stop=True)
            for c in range(NCK):
                # rbc lives in PSUM -> DVE only (GPSIMD can't read PSUM)
                nc.vector.tensor_tensor(xh1[c][:], x1[c][:], rbc[:], ALU.mult)
                nc.vector.tensor_tensor(xh18[:, c, :], x1[c][:], rbc[:],
                                        ALU.mult)

        # ---------------- phase 5: FFN ----------------------------------
        with tc.tile_pool(name="ffn", bufs=3) as ffn:
            for half in range(2):
                with tc.tile_pool(name=f"y_ps{half}", bufs=1,
                                  space="PSUM") as y_ps, \
                     tc.tile_pool(name=f"f_psh{half}", bufs=2,
                                  space="PSUM") as f_psh, \
                     tc.tile_pool(name=f"f_ps{half}", bufs=2,
                                  space="PSUM") as f_ps:
                    yps = y_ps.tile([128, 3 * QB], F32, name="y")

                    def w2_acc(mj, half=half, yps=yps):
                        for o3 in range(3):
                            oc = half * 3 + o3
                            nc.tensor.matmul(
                                yps[:, o3 * QB:(o3 + 1) * QB],
                                w2_sb[mj][:, :, oc * 128:(oc + 1) * 128],
                                u2[mj][:], start=(mj == 0), stop=(mj == 11),
                                perf_mode=DR, skip_group_check=True)

                    for j in range(24):
                        if half == 0:
                            if j in pre_w1:
                                w1a = pre_w1[j]
                            else:
                                w1a = w1s.tile([128, NCK, 128], BF16, name="w1a")
                                nc.sync.dma_start(w1a[:], w1t_d[j])
                            psg = f_ps.tile([128, QB], F32, name="psg")
                            psh = f_psh.tile([128, QB], F32, name="psh")
                            # psh first: the sigmoid chain hangs off it
                            for c in range(NCK):
                                nc.tensor.matmul(psh[:], w1a[:, c, :], xh1[c][:],
                                                 start=(c == 0),
                                                 stop=(c == NCK - 1))
                            for m in range(NCK // 2):
                                nc.tensor.matmul(
                                    psg[:],
                                    w1g_sb[:, m, :, j * 128:(j + 1) * 128],
                                    xh18[:, 2 * m:2 * m + 2, :],
                                    start=(m == 0), stop=(m == NCK // 2 - 1),
                                    perf_mode=DR)
                            # w2 matmuls for the previous pair: gives the
                            # cross-engine pointwise chain a full pair-period
                            # to finish before the PE needs u2[mj]
                            if j % 2 == 0 and j >= 2:
                                w2_acc(j // 2 - 1)
                            # mish(x)*g ~= SIG_C*x*sigmoid(SIG_A x + SIG_B)*g
                            # (SIG_C folded into w2 on host)
                            sg = ffn.tile([128, QB], BF16, name="sg")
                            nc.scalar.activation(out=sg[:], in_=psh[:],
                                                 func=AF.Sigmoid,
                                                 scale=SIG_A, bias=sigb[:])
                            G = ffn.tile([128, QB], BF16, name="mish_G")
                            nc.vector.tensor_tensor(G[:], psh[:], sg[:],
                                                    ALU.mult)
                            nc.vector.tensor_tensor(u2[j // 2][:, j % 2, :],
                                                    G[:], psg[:], ALU.mult)
                        else:
                            if j % 2 == 0:
                                w2_acc(j // 2)
                    if half == 0:
                        w2_acc(11)
                    for o3 in range(3):
                        oc = half * 3 + o3
                        yout = ffn.tile([128, QB], BF16, name="yout")
                        # fold the 1/A2 w2-scale into the residual add
                        nc.vector.scalar_tensor_tensor(
                            yout[:], yps[:, o3 * QB:(o3 + 1) * QB], 1.0 / A2,
                            x1[oc][:], ALU.mult, ALU.add)
                        nc.sync.dma_start(
                            yT_d[oc * 128:(oc + 1) * 128, :], yout[:])


# ---------------------------------------------------------------------------
# Host wrapper
# ---------------------------------------------------------------------------

_NC_CACHE = {}


def _get_nc():
    if "nc" not in _NC_CACHE:
        _NC_CACHE["nc"] = build_nc()
    return _NC_CACHE["nc"]


def _prep_inputs(x, w_qkv, w_out, w1, w2, g_attn, g_ff):
    bf16 = ml_dtypes.bfloat16
    f8 = ml_dtypes.float8_e4m3fn
    scale = 1.0 / np.sqrt(HD)
    wqkv_f = (g_attn[:, None] * w_qkv).astype(np.float32).copy()
    wqkv_f[:, : 2 * DIM] *= scale * AQK  # attn scale + fp8 range scale (q, k)
    wqkv_f[:, 2 * DIM:] *= AV            # fp8 range scale (v)
    # fp8 d-chunk-pair layout: [p, m, s, col] = w[(2m+s)*128+p, col]
    wqkv8 = np.ascontiguousarray(
        wqkv_f.reshape(3, 2, 128, 3 * DIM).transpose(2, 0, 1, 3)
        .astype(f8)).view(np.uint8)
    wout_b = np.ascontiguousarray(w_out.astype(bf16))
    w1_f = (g_ff[:, None] * w1).astype(np.float32)
    # h-half pre-tiled: [768,3072] -> [24, 128, 6, 128]
    w1t = np.ascontiguousarray(
        w1_f[:, :HIDDEN].reshape(NCK, 128, 24, 128)
        .transpose(2, 1, 0, 3).astype(bf16))
    # gate half fp8 (AG-scaled), d-chunk-pair layout [p, m, s, col]
    w1g8 = np.ascontiguousarray(
        (w1_f[:, HIDDEN:] * AG).reshape(3, 2, 128, HIDDEN)
        .transpose(2, 0, 1, 3).astype(f8)).view(np.uint8)
    # w2 fp8 hidden-chunk-pair layout: [p, mj, s, col] = w2[(2mj+s)*128+p, col]
    # (SIG_C from the sigmoid fit and 1/AG from the gate scale fold in here)
    w28 = np.ascontiguousarray(
        (w2 * (A2 * SIG_C / AG)).astype(np.float32)
        .reshape(12, 2, 128, DIM).transpose(2, 0, 1, 3).astype(f8)
        ).view(np.uint8)

    in_maps = []
    for core in range(8):
        b, qb = core // 4, core % 4
        xb = np.ascontiguousarray(
            np.roll(x[b], -qb * QB, axis=0).T.astype(bf16))
        in_maps.append({
            "xb": xb,
            "wqkv8": wqkv8,
            "wout": wout_b,
            "w1t": w1t,
            "w1g8": w1g8,
            "w28": w28,
        })
    return in_maps


def run(x, w_qkv, w_out, w1, w2, g_attn, g_ff, trace=False, **kw):
    nc = _get_nc()
    in_maps = _prep_inputs(x, w_qkv, w_out, w1, w2, g_attn, g_ff)
    res = run_bass_kernel_spmd(
        nc, in_maps, core_ids=list(range(8)), trace=trace, **kw)
    B = x.shape[0]
    y = np.zeros((B, S, DIM), dtype=np.float32)
    for core in range(8):
        b, qb = core // 4, core % 4
        yT = res.results[core]["yT"]  # [768, 512] bf16
        y[b, qb * QB:(qb + 1) * QB, :] = np.asarray(yT).astype(np.float32).T
    return y, res


def kernel(x, w_qkv, w_out, w1, w2, g_attn, g_ff):
    y, _ = run(np.asarray(x, np.float32), np.asarray(w_qkv, np.float32),
               np.asarray(w_out, np.float32), np.asarray(w1, np.float32),
               np.asarray(w2, np.float32), np.asarray(g_attn, np.float32),
               np.asarray(g_ff, np.float32))
    return y
